# revision 16
# baseline (speedup 1.0000x reference)
"""Trainium2 Bass kernel for ContextQueryAttn (BiDAF-style trilinear attention).

Computes, per batch b (context compacted to the NC2 unmasked rows):
    cross = (ctx*wm) @ query.T                  (NC2, Lq)
    Pc  = exp(cross)            [c, q]          (one EXP, the only one)
    PT  = blocked-transpose(Pc) [q, (ci,qt), c] via ONE xbar-DMA per EXP group
    T   = Pc^T @ (e^{sc}[ctx|1])  -> col-softmax numerator + normalizer
    Tn' = T * (e^{sqb}/colsum)  [q, d]          (e^{sqb}=exp(sq+qmask bias))
    A|B = PT^T @ [e^{sqb}qe | Tn']  (NC2, 512)  one 512-col matmul per tile
    rs  = PT^T @ e^{sqb}            (NC2,)      row-softmax normalizer
    cm  = (e^{-sq}mq)^T @ Tn'       (1, d)      sum of Tn over unmasked q
Host divides A|B by rs; masked-context rows from cm / query-mean.
Per-row e^{sc} factors cancel in the row softmax; per-col e^{sqb} factors are
folded into qe (host), the ones column (host) and Tn (device), so a single
exponentiation of the sim matrix suffices.

Engine layout: PE streams Pc/T/A|B matmuls; ACT only does 3 EXPs + a small
drain per batch; DVE drains bf16 A|B psum at 2x; xbar transposes and DMA
dispatches are spread over Sync/Scalar/GpSimd sequencers (HWDGE dispatch costs
~630ns each, so dispatch count per engine matters).
"""

import numpy as np
import ml_dtypes

import concourse.bass as bass
import concourse.tile as tile
from concourse import bacc, mybir
from concourse.bass_utils import run_bass_kernel_spmd

F32 = mybir.dt.float32
BF16 = mybir.dt.bfloat16
NPBF16 = ml_dtypes.bfloat16
EXP = mybir.ActivationFunctionType.Exp

B, LC, LQ, D = 32, 2048, 256, 256
NCORES = 8
BPC = B // NCORES          # batches per core
BG = BPC // 2              # 2-batch load groups per core
NKD = D // 128             # 2 contraction chunks over D
NQT = LQ // 128            # 2 query tiles
NEG = np.float32(-1e30)


def _build_kernel(tc, nc, ins, outs, NC2):
    import contextlib
    NT = NC2 // 128
    ctx = contextlib.ExitStack()

    sb = lambda name, bufs: ctx.enter_context(tc.tile_pool(name=name, bufs=bufs))
    psA = ctx.enter_context(tc.tile_pool(name="psA", bufs=2, space="PSUM"))
    psAB = ctx.enter_context(tc.tile_pool(name="psAB", bufs=2, space="PSUM"))
    psT = ctx.enter_context(tc.tile_pool(name="psT", bufs=4, space="PSUM"))

    p_ctxT = sb("pctxT", 2)
    p_ctx = sb("pctx", 2)
    p_qwm = sb("pqwm", 2)
    p_esq = sb("pesq", 2)
    p_mq = sb("pmq", 2)
    p_qeTn = sb("pqeTn", BPC)
    p_pt = sb("ppt", 2)
    p_pc = sb("ppc", 2)
    p_cs = sb("pcs", 2)
    p_ast = sb("past", 2)
    p_rs = sb("prs", 2)
    p_cm = sb("pcm", 2)

    pc_groups = []
    i = 0
    while i < NT:
        pc_groups.append((i, min(2, NT - i)))
        i += 2

    # ---- all loads dispatched up front on the SWDGE queue, ordered by
    #      first use, with the bulk tensors split per batch so the FIFO
    #      delivers each batch's data just in time ----
    group_tiles = {}
    qeTn_tiles = {}
    for g in range(BG):
        qwm_sb = p_qwm.tile([128, 2, NKD, LQ], BF16, name="qwm_sb")
        nc.gpsimd.dma_start(out=qwm_sb[:], in_=ins["qwm2"][g])
        esq_sb = p_esq.tile([128, 2, NQT], F32, name="esq_sb")
        nc.gpsimd.dma_start(out=esq_sb[:], in_=ins["esq"][g])
        mq_sb = p_mq.tile([128, 2, NQT], BF16, name="mq_sb")
        nc.gpsimd.dma_start(out=mq_sb[:], in_=ins["mq2"][g])
        ctxT_sb = p_ctxT.tile([128, 2, NKD, NC2], BF16, name="ctxT_sb")
        ctx_sb = p_ctx.tile([128, 2, NT, 258], BF16, name="ctx_sb")
        group_tiles[g] = dict(ctxT=ctxT_sb, ctx=ctx_sb, qwm=qwm_sb,
                              esq=esq_sb, mq=mq_sb)
        for jj in range(2):
            b = 2 * g + jj
            nc.gpsimd.dma_start(out=ctxT_sb[:, jj], in_=ins["ctxT2"][g][:, jj])
            nc.gpsimd.dma_start(out=ctx_sb[:, jj], in_=ins["ctx2"][g][:, jj])
            qeTn = p_qeTn.tile([128, NQT, 513], BF16, name="qeTn")
            nc.gpsimd.dma_start(out=qeTn[:], in_=ins["qe2"][b])
            qeTn_tiles[b] = qeTn

    for b in range(BPC):
        g, j = divmod(b, 2)
        ctxT_sb = group_tiles[g]["ctxT"]
        ctx_sb = group_tiles[g]["ctx"]
        qwm_sb = group_tiles[g]["qwm"]
        esq_sb = group_tiles[g]["esq"]
        mq_sb = group_tiles[g]["mq"]
        # per-batch: [e^{sqb}qe(256) | Tn'(256) | e^{sqb}(1)] rhs image
        qeTn = qeTn_tiles[b]

        # ---- phase 1: Pc = exp(cross) [c', q]; PT = xbar block transpose;
        #      T accumulation braided behind the Pc EXPs ----
        # PT layout: [q-lane, m=(ci*NQT+qt), c-lane] per xbar semantics.
        PT_sb = p_pt.tile([128, NT * NQT, 128], BF16, name="PT_sb")
        Pc_sb = p_pc.tile([128, NT * LQ], BF16, name="Pc_sb")
        T_ps = [psT.tile([128, 512], F32, tag="psT", name=f"T_ps{qt}")
                for qt in range(NQT)]

        def emit_pc(gi):
            ci0, w = pc_groups[gi]
            psc = psA.tile([128, 512], F32, tag="psA", name="psc")
            for k in range(w):
                for kd in range(NKD):
                    nc.tensor.matmul(
                        psc[:, k * LQ:(k + 1) * LQ],
                        lhsT=ctxT_sb[:, j, kd, bass.ts(ci0 + k, 128)],
                        rhs=qwm_sb[:, j, kd, :],
                        start=(kd == 0), stop=(kd == NKD - 1))
            nc.scalar.activation(
                Pc_sb[:, ci0 * LQ:(ci0 + w) * LQ], psc[:, 0:w * LQ], EXP)
            # one dispatch transposes the whole EXP group into PT blocks
            # (the Sync ring carries ONLY transposes, so they never queue
            # behind bulk transfers and never block ACT's FIFO)
            nc.sync.dma_start_transpose(
                out=PT_sb[:, ci0 * NQT:(ci0 + w) * NQT, :],
                in_=Pc_sb[:, ci0 * LQ:(ci0 + w) * LQ])

        def emit_t(ci):
            for qt in range(NQT):
                nc.tensor.matmul(
                    T_ps[qt][:, 0:258],
                    lhsT=Pc_sb[:, ci * LQ + qt * 128:ci * LQ + qt * 128 + 128],
                    rhs=ctx_sb[:, j, ci, :],
                    start=(ci == 0), stop=(ci == NT - 1))

        npc = len(pc_groups)
        tq = []
        for i in range(npc + 1):
            if i < npc:
                emit_pc(i)
            if i >= 1:
                ci0, w = pc_groups[i - 1]
                tq.extend(range(ci0, ci0 + w))
            while len(tq) > 4:
                emit_t(tq.pop(0))
        for ci in tq:
            emit_t(ci)

        # ---- T finalize: Tn' = T * (e^{sqb}/colsum) into the qeTn image ----
        csrec = p_cs.tile([128, 2 * NQT], F32, name="csrec")
        for qt in range(NQT):
            nc.vector.reciprocal(csrec[:, qt:qt + 1], T_ps[qt][:, 256:257])
        nc.vector.tensor_mul(csrec[:, NQT:2 * NQT], csrec[:, 0:NQT],
                             esq_sb[:, j, :])
        for qt in range(NQT):
            nc.vector.tensor_scalar_mul(
                qeTn[:, qt, 256:512], T_ps[qt][:, 0:256],
                csrec[:, NQT + qt:NQT + qt + 1])

        # ---- phase 2: A|B numerators in bf16 psum, rowsum + cm sideband ----
        ABst = p_ast.tile([128, NT * 512], BF16, name="ABst")
        rs_st = p_rs.tile([128, 16], F32, name="rs_st")
        cm_st = p_cm.tile([128, 256], F32, name="cm_st")
        rsm = psT.tile([128, 512], F32, tag="psT", name="rsm")

        for ci in range(NT):
            pab = psAB.tile([128, 512], F32, tag="psAB", name="pab")
            for qt in range(NQT):
                nc.tensor.matmul(
                    pab[:, 0:512],
                    lhsT=PT_sb[:, ci * NQT + qt, :],
                    rhs=qeTn[:, qt, 0:512],
                    start=(qt == 0), stop=(qt == NQT - 1))
                nc.tensor.matmul(
                    rsm[:, ci:ci + 1],
                    lhsT=PT_sb[:, ci * NQT + qt, :],
                    rhs=qeTn[:, qt, 512:513],
                    start=(qt == 0), stop=(qt == NQT - 1))
            # drains on DVE only: ACT's strict FIFO must stay clear for the
            # next batch's EXPs (head-of-line coupling across batches)
            nc.vector.tensor_copy(
                ABst[:, ci * 512:(ci + 1) * 512], pab[:, 0:512])

        # cm = sum of Tn rows over unmasked q (masked-context B rows)
        for qt in range(NQT):
            nc.tensor.matmul(
                rsm[0:1, 256:512],
                lhsT=mq_sb[:, j, qt:qt + 1],
                rhs=qeTn[:, qt, 256:512],
                start=(qt == 0), stop=(qt == NQT - 1))

        nc.vector.tensor_copy(rs_st[:, 0:NT], rsm[:, 0:NT])
        nc.vector.tensor_copy(cm_st[0:1, :], rsm[0:1, 256:512])

        nc.gpsimd.dma_start(out=outs["ABo"][b], in_=ABst[:])
        nc.gpsimd.dma_start(out=outs["rso"][b], in_=rs_st[:])
        nc.gpsimd.dma_start(out=outs["cmo"][b], in_=cm_st[0:1, :])

    ctx.close()


def build_program(NC2):
    NT = NC2 // 128
    nc = bacc.Bacc("TRN2", target_bir_lowering=False, debug=False,
                   num_devices=NCORES)
    ins = {
        "ctxT2": nc.dram_tensor("ctxT2", [BG, 128, 2, NKD, NC2], BF16,
                                kind="ExternalInput").ap(),
        "ctx2": nc.dram_tensor("ctx2", [BG, 128, 2, NT, 258], BF16,
                               kind="ExternalInput").ap(),
        "qwm2": nc.dram_tensor("qwm2", [BG, 128, 2, NKD, LQ], BF16,
                               kind="ExternalInput").ap(),
        "esq": nc.dram_tensor("esq", [BG, 128, 2, NQT], F32,
                              kind="ExternalInput").ap(),
        "mq2": nc.dram_tensor("mq2", [BG, 128, 2, NQT], BF16,
                              kind="ExternalInput").ap(),
        "qe2": nc.dram_tensor("qe2", [BPC, 128, NQT, 513], BF16,
                              kind="ExternalInput").ap(),
    }
    outs = {
        "ABo": nc.dram_tensor("ABo", [BPC, 128, NT * 512], BF16,
                              kind="ExternalOutput").ap(),
        "rso": nc.dram_tensor("rso", [BPC, 128, 16], F32,
                              kind="ExternalOutput").ap(),
        "cmo": nc.dram_tensor("cmo", [BPC, 1, 256], F32,
                              kind="ExternalOutput").ap(),
    }
    with tile.TileContext(nc) as tc:
        _build_kernel(tc, nc, ins, outs, NC2)
    nc.compile()
    return nc


def _aux(context_mask):
    """Per-batch unmasked-context indices and the padded compact size."""
    cm = np.asarray(context_mask).astype(bool)
    idx = [np.flatnonzero(~cm[b]) for b in range(cm.shape[0])]
    nmax = max((len(u) for u in idx), default=1)
    NC2 = max(256, ((int(nmax) + 127) // 128) * 128)
    return idx, NC2


def _img(a, p=128):
    """[N*p, X...] row-major -> SBUF image [p, N, X...] (row r = t*p + lane)."""
    n = a.shape[0] // p
    return np.ascontiguousarray(
        a.reshape((n, p) + a.shape[1:]).swapaxes(0, 1))


def host_prep(context, query, context_mask, query_mask, w0):
    """Host-side preprocessing: compact, shard, build device blobs."""
    f = np.float32
    context = np.asarray(context, dtype=f)
    query = np.asarray(query, dtype=f)
    w0 = np.asarray(w0, dtype=f)
    wc, wq, wm = w0[:D], w0[D:2 * D], w0[2 * D:]
    qmf = np.asarray(query_mask).astype(f)                  # (B, LQ)
    idx, NC2 = _aux(context_mask)
    NT = NC2 // 128

    sq = query @ wq                                         # (B, LQ)
    esq = ((1.0 - qmf) * np.exp(sq, dtype=f)).astype(f)     # 0 on masked q
    emq = ((1.0 - qmf) * np.exp(-sq, dtype=f)).astype(f)    # e^{-sq}, 0 masked
    qwmT = (query * wm).transpose(0, 2, 1)                  # (B, D, LQ) f32
    qe = np.zeros((B, LQ, 513), f)
    qe[:, :, 0:256] = query * esq[:, :, None]
    qe[:, :, 512] = esq

    in_maps = []
    for c in range(NCORES):
        m = {"ctxT2": np.zeros((BG, 128, 2, NKD, NC2), NPBF16),
             "ctx2": np.zeros((BG, 128, 2, NT, 258), NPBF16),
             "qwm2": np.empty((BG, 128, 2, NKD, LQ), NPBF16),
             "esq": np.zeros((BG, 128, 2, NQT), f),
             "mq2": np.zeros((BG, 128, 2, NQT), NPBF16),
             "qe2": np.empty((BPC, 128, NQT, 513), NPBF16)}
        for lb in range(BPC):
            b = c * BPC + lb
            g, jj = divmod(lb, 2)
            U = idx[b]
            n = len(U)
            cU = context[b][U]                              # (n, D)
            scU = cU @ wc                                   # (n,)
            ctxT_pad = np.zeros((D, NC2), f)
            ctxT_pad[:, :n] = cU.T
            m["ctxT2"][g, :, jj] = _img(ctxT_pad).astype(NPBF16)
            # ctx rows scaled by e^{sc[c]} (column-softmax weight); the
            # ones-col picks up the same factor => correct normalizer.
            ctx_pad = np.zeros((NC2, 258), f)
            ctx_pad[:n, :D] = cU
            ctx_pad[:n, D] = 1.0
            ctx_pad[:n] *= np.exp(scU, dtype=f)[:, None]
            m["ctx2"][g, :, jj] = _img(ctx_pad).astype(NPBF16)
            m["qwm2"][g, :, jj] = _img(qwmT[b]).astype(NPBF16)
            m["esq"][g, :, jj] = esq[b].reshape(NQT, 128).T
            m["mq2"][g, :, jj] = emq[b].reshape(NQT, 128).T
            m["qe2"][lb] = _img(qe[b]).astype(NPBF16)
        in_maps.append(m)
    return in_maps


_cached_nc = {}


def get_program(NC2):
    if NC2 not in _cached_nc:
        _cached_nc[NC2] = build_program(NC2)
    return _cached_nc[NC2]


def run_on_hw(in_maps, **kwargs):
    NC2 = in_maps[0]["ctxT2"].shape[-1]
    nc = get_program(NC2)
    return run_bass_kernel_spmd(nc, in_maps, core_ids=list(range(NCORES)),
                                **kwargs)


def kernel(context, query, context_mask, query_mask, w0):
    f = np.float32
    context = np.asarray(context, dtype=f)
    query = np.asarray(query, dtype=f)
    w0 = np.asarray(w0, dtype=f)
    qmask = np.asarray(query_mask).astype(bool)
    idx, NC2 = _aux(context_mask)
    NT = NC2 // 128
    ctxmean = context.mean(1, dtype=np.float64).astype(f)   # (B, D)
    in_maps = host_prep(context, query, context_mask, query_mask, w0)
    res = run_on_hw(in_maps)

    A = np.empty((B, LC, D), f)
    Bm = np.empty((B, LC, D), f)
    cmask = np.asarray(context_mask).astype(bool)
    for c in range(NCORES):
        r = res.results[c]
        for lb in range(BPC):
            b = c * BPC + lb
            U = idx[b]
            n = len(U)
            ABr = r["ABo"][lb].astype(f).reshape(128, NT, 512).swapaxes(0, 1)
            ABr = ABr.reshape(NC2, 512)
            rs = r["rso"][lb][:, :NT].astype(f).T.reshape(NC2)
            cm = r["cmo"][lb][0].astype(f)                  # (256,)
            inv = 1.0 / rs[:n]
            A[b][U] = ABr[:n, 0:256] * inv[:, None]
            Bm[b][U] = ABr[:n, 256:512] * inv[:, None]
            nmq = float(qmask[b].sum())
            colmean = (cm + nmq * ctxmean[b]) / np.float32(LQ)
            mrow = cmask[b]
            A[b][mrow] = query[b].mean(0, dtype=np.float64).astype(f)
            Bm[b][mrow] = colmean
    return A, Bm


# revision 17
# speedup vs baseline: 1.9291x; 1.9291x over previous
"""Trainium2 Bass kernel for ContextQueryAttn (BiDAF-style trilinear attention).

Computes, per batch b (context compacted to the NC2 unmasked rows):
    sim = sc[:,None] + sq[None,:] + (ctx*wm) @ query.T          (NC2, Lq)
    PT  = exp(sim^T + sq_bias[q])    [q, c]   (zero rows for masked q)
    Pc  = exp(cross)                 [c, q]   (e^{sc} folded into ctx rows)
    T   = Pc^T @ (e^{sc}[ctx|1])  -> col-softmax numerator + normalizer
    Tn  = T / colsum                 [q, d]
    A|B = PT^T @ [qe | Tn]           (NC2, 512)  one 512-col matmul per tile
    rs  = PT^T @ 1                   (NC2,)      row-softmax normalizer
    cm  = mq^T @ Tn                  (1, d)      sum of Tn over unmasked q
Host divides A|B by rs, reconstructs masked rows from cm / query-mean.

Scheduling notes (the part that matters for performance):
 - Every engine executes its instruction queue in EMISSION order, so the
   batch loop is software-pipelined: phase 1 of batch b+1 is emitted
   BEFORE phase 2 of batch b.  Without this, each batch's serial chain
   (load -> Pc matmul -> EXP -> T -> Tn -> A|B -> drain -> store)
   serializes the whole kernel (~19us x 4 batches).
 - All loads are dispatched up front on the SWDGE (GpSimd) queue in
   first-use order (HWDGE/SWDGE dispatch costs ~0.6-1us of sequencer time
   each, and each queue is FIFO).  Stores ride the Sync queue.
 - ACT runs the EXPs plus one A|B drain per batch; DVE takes the rest.
"""

import numpy as np
import ml_dtypes

import concourse.bass as bass
import concourse.tile as tile
from concourse import bacc, mybir
from concourse.bass_utils import run_bass_kernel_spmd

F32 = mybir.dt.float32
BF16 = mybir.dt.bfloat16
NPBF16 = ml_dtypes.bfloat16
EXP = mybir.ActivationFunctionType.Exp

B, LC, LQ, D = 32, 2048, 256, 256
NCORES = 8
BPC = B // NCORES          # batches per core
BG = BPC // 2              # 2-batch load groups per core
NKD = D // 128             # 2 contraction chunks over D
NQT = LQ // 128            # 2 query tiles
NEG = np.float32(-1e30)


def _build_kernel(tc, nc, ins, outs, NC2):
    import contextlib
    NT = NC2 // 128
    ctx = contextlib.ExitStack()

    sb = lambda name, bufs: ctx.enter_context(tc.tile_pool(name=name, bufs=bufs))
    psA = ctx.enter_context(tc.tile_pool(name="psA", bufs=3, space="PSUM"))
    psT = ctx.enter_context(tc.tile_pool(name="psT", bufs=2, space="PSUM"))

    p_ctxT = sb("pctxT", 2)
    p_ctx = sb("pctx", 2)
    p_qwm = sb("pqwm", 2)
    p_fv = sb("pfv", 2)
    p_mq = sb("pmq", 2)
    p_qeTn = sb("pqeTn", BPC)
    p_pt = sb("ppt", 2)
    p_pc = sb("ppc", 2)
    p_cs = sb("pcs", 2)
    p_ast = sb("past", 2)
    p_rs = sb("prs", 2)
    p_cm = sb("pcm", 2)

    pt_chunks = []
    off = 0
    while off < NC2:
        w = min(1024, NC2 - off)
        pt_chunks.append((off, w))
        off += w
    pc_groups = []
    i = 0
    while i < NT:
        pc_groups.append((i, min(4, NT - i)))
        i += 4

    # ---- all loads up front on the SWDGE queue, in first-use order ----
    group_tiles = {}
    qeTn_tiles = {}
    for g in range(BG):
        qwm_sb = p_qwm.tile([128, 2, NKD, LQ], BF16, name="qwm_sb")
        nc.gpsimd.dma_start(out=qwm_sb[:], in_=ins["qwm2"][g])
        fv_sb = p_fv.tile([128, 2, NQT], F32, name="fv_sb")
        nc.gpsimd.dma_start(out=fv_sb[:], in_=ins["fvec"][g])
        mq_sb = p_mq.tile([128, 2, NQT], BF16, name="mq_sb")
        nc.gpsimd.dma_start(out=mq_sb[:], in_=ins["mq"][g])
        ctxT_sb = p_ctxT.tile([128, 2, NKD, NC2], BF16, name="ctxT_sb")
        ctx_sb = p_ctx.tile([128, 2, NT, 258], BF16, name="ctx_sb")
        group_tiles[g] = dict(ctxT=ctxT_sb, ctx=ctx_sb, qwm=qwm_sb,
                              fv=fv_sb, mq=mq_sb)
        for jj in range(2):
            b = 2 * g + jj
            nc.gpsimd.dma_start(out=ctxT_sb[:, jj], in_=ins["ctxT2"][g][:, jj])
            nc.gpsimd.dma_start(out=ctx_sb[:, jj], in_=ins["ctx2"][g][:, jj])
            qeTn = p_qeTn.tile([128, NQT, 513], BF16, name="qeTn")
            nc.gpsimd.dma_start(out=qeTn[:], in_=ins["qe2"][b])
            qeTn_tiles[b] = qeTn

    state = {}

    def emit_phase1(b):
        g, j = divmod(b, 2)
        ctxT_sb = group_tiles[g]["ctxT"]
        ctx_sb = group_tiles[g]["ctx"]
        qwm_sb = group_tiles[g]["qwm"]
        fv_sb = group_tiles[g]["fv"]
        qeTn = qeTn_tiles[b]

        PT_sb = p_pt.tile([128, NQT, NC2], BF16, name="PT_sb")
        Pc_sb = p_pc.tile([128, NT * LQ], BF16, name="Pc_sb")
        T_ps = [psT.tile([128, 512], F32, tag="psT", name=f"T_ps{qt}")
                for qt in range(NQT)]

        def emit_pt(qt, off, w):
            ps = psA.tile([128, 1024], F32, tag="psA", name="ps_pt")
            o2 = 0
            while o2 < w:
                cw = min(512, w - o2)
                for kd in range(NKD):
                    nc.tensor.matmul(
                        ps[:, o2:o2 + cw],
                        lhsT=qwm_sb[:, j, kd, bass.ts(qt, 128)],
                        rhs=ctxT_sb[:, j, kd, off + o2:off + o2 + cw],
                        start=(kd == 0), stop=(kd == NKD - 1))
                o2 += cw
            nc.scalar.activation(
                PT_sb[:, qt, off:off + w], ps[:, 0:w], EXP,
                bias=fv_sb[:, j, qt:qt + 1])

        def emit_pc(gi):
            ci0, w = pc_groups[gi]
            psc = psA.tile([128, 1024], F32, tag="psA", name="psc")
            for k in range(w):
                for kd in range(NKD):
                    nc.tensor.matmul(
                        psc[:, k * LQ:(k + 1) * LQ],
                        lhsT=ctxT_sb[:, j, kd, bass.ts(ci0 + k, 128)],
                        rhs=qwm_sb[:, j, kd, :],
                        start=(kd == 0), stop=(kd == NKD - 1))
            nc.scalar.activation(
                Pc_sb[:, ci0 * LQ:(ci0 + w) * LQ], psc[:, 0:w * LQ], EXP)

        def emit_t(ci):
            for qt in range(NQT):
                nc.tensor.matmul(
                    T_ps[qt][:, 0:258],
                    lhsT=Pc_sb[:, ci * LQ + qt * 128:ci * LQ + qt * 128 + 128],
                    rhs=ctx_sb[:, j, ci, :],
                    start=(ci == 0), stop=(ci == NT - 1))

        pt_units = [(qt, off, w) for qt in range(NQT)
                    for (off, w) in pt_chunks]
        npc = len(pc_groups)
        tq = []
        for i in range(max(len(pt_units), npc + 1)):
            if i < len(pt_units):
                emit_pt(*pt_units[i])
            if i < npc:
                emit_pc(i)
            if i >= 1 and i - 1 < npc:
                ci0, w = pc_groups[i - 1]
                tq.extend(range(ci0, ci0 + w))
            while len(tq) > 4:
                emit_t(tq.pop(0))
        for ci in tq:
            emit_t(ci)

        # Tn = T / colsum into the qeTn rhs image
        csrec = p_cs.tile([128, NQT], F32, name="csrec")
        for qt in range(NQT):
            nc.vector.reciprocal(csrec[:, qt:qt + 1], T_ps[qt][:, 256:257])
            nc.vector.tensor_scalar_mul(
                qeTn[:, qt, 256:512], T_ps[qt][:, 0:256],
                csrec[:, qt:qt + 1])
        state[b] = dict(PT=PT_sb, T_ps=T_ps)

    def emit_phase2(b):
        g, j = divmod(b, 2)
        mq_sb = group_tiles[g]["mq"]
        qeTn = qeTn_tiles[b]
        PT_sb = state[b]["PT"]

        ABst = p_ast.tile([128, NT * 512], BF16, name="ABst")
        rs_st = p_rs.tile([128, 16], F32, name="rs_st")
        cm_st = p_cm.tile([128, 256], F32, name="cm_st")
        rsm = psT.tile([128, 512], F32, tag="psT", name="rsm")

        pi = 0
        for ci0 in range(0, NT, 2):
            w = min(2, NT - ci0)
            pab = psA.tile([128, 1024], F32, tag="psA", name="pab")
            for k in range(w):
                ci = ci0 + k
                for qt in range(NQT):
                    nc.tensor.matmul(
                        pab[:, k * 512:k * 512 + 512],
                        lhsT=PT_sb[:, qt, bass.ts(ci, 128)],
                        rhs=qeTn[:, qt, 0:512],
                        start=(qt == 0), stop=(qt == NQT - 1))
                    nc.tensor.matmul(
                        rsm[:, ci:ci + 1],
                        lhsT=PT_sb[:, qt, bass.ts(ci, 128)],
                        rhs=qeTn[:, qt, 512:513],
                        start=(qt == 0), stop=(qt == NQT - 1))
            # one drain per psum tile; ACT takes a single drain per batch to
            # stay mostly free for the (already-emitted) next batch's EXPs
            if pi == 1:
                nc.scalar.copy(
                    ABst[:, ci0 * 512:(ci0 + w) * 512], pab[:, 0:w * 512])
            else:
                nc.vector.tensor_copy(
                    ABst[:, ci0 * 512:(ci0 + w) * 512], pab[:, 0:w * 512])
            pi += 1

        # cm = sum of Tn rows over unmasked q (masked-context B rows)
        for qt in range(NQT):
            nc.tensor.matmul(
                rsm[0:1, 256:512],
                lhsT=mq_sb[:, j, qt:qt + 1],
                rhs=qeTn[:, qt, 256:512],
                start=(qt == 0), stop=(qt == NQT - 1))

        nc.vector.tensor_copy(rs_st[:, 0:NT], rsm[:, 0:NT])
        nc.vector.tensor_copy(cm_st[0:1, :], rsm[0:1, 256:512])

        nc.sync.dma_start(out=outs["ABo"][b], in_=ABst[:])
        nc.sync.dma_start(out=outs["rso"][b], in_=rs_st[:])
        nc.sync.dma_start(out=outs["cmo"][b], in_=cm_st[0:1, :])

    # software pipeline: phase 1 of b+1 is emitted before phase 2 of b
    for b in range(BPC + 1):
        if b < BPC:
            emit_phase1(b)
        if b >= 1:
            emit_phase2(b - 1)

    ctx.close()


def build_program(NC2):
    NT = NC2 // 128
    nc = bacc.Bacc("TRN2", target_bir_lowering=False, debug=False,
                   num_devices=NCORES)
    ins = {
        "ctxT2": nc.dram_tensor("ctxT2", [BG, 128, 2, NKD, NC2], BF16,
                                kind="ExternalInput").ap(),
        "ctx2": nc.dram_tensor("ctx2", [BG, 128, 2, NT, 258], BF16,
                               kind="ExternalInput").ap(),
        "qwm2": nc.dram_tensor("qwm2", [BG, 128, 2, NKD, LQ], BF16,
                               kind="ExternalInput").ap(),
        "fvec": nc.dram_tensor("fvec", [BG, 128, 2, NQT], F32,
                               kind="ExternalInput").ap(),
        "mq": nc.dram_tensor("mq", [BG, 128, 2, NQT], BF16,
                             kind="ExternalInput").ap(),
        "qe2": nc.dram_tensor("qe2", [BPC, 128, NQT, 513], BF16,
                              kind="ExternalInput").ap(),
    }
    outs = {
        "ABo": nc.dram_tensor("ABo", [BPC, 128, NT * 512], BF16,
                              kind="ExternalOutput").ap(),
        "rso": nc.dram_tensor("rso", [BPC, 128, 16], F32,
                              kind="ExternalOutput").ap(),
        "cmo": nc.dram_tensor("cmo", [BPC, 1, 256], F32,
                              kind="ExternalOutput").ap(),
    }
    with tile.TileContext(nc) as tc:
        _build_kernel(tc, nc, ins, outs, NC2)
    nc.compile()
    return nc


def _aux(context_mask):
    """Per-batch unmasked-context indices and the padded compact size."""
    cm = np.asarray(context_mask).astype(bool)
    idx = [np.flatnonzero(~cm[b]) for b in range(cm.shape[0])]
    nmax = max((len(u) for u in idx), default=1)
    NC2 = max(256, ((int(nmax) + 127) // 128) * 128)
    return idx, NC2


def _img(a, p=128):
    """[N*p, X...] row-major -> SBUF image [p, N, X...] (row r = t*p + lane)."""
    n = a.shape[0] // p
    return np.ascontiguousarray(
        a.reshape((n, p) + a.shape[1:]).swapaxes(0, 1))


def host_prep(context, query, context_mask, query_mask, w0):
    """Host-side preprocessing: compact, shard, build device blobs."""
    f = np.float32
    context = np.asarray(context, dtype=f)
    query = np.asarray(query, dtype=f)
    w0 = np.asarray(w0, dtype=f)
    wc, wq, wm = w0[:D], w0[D:2 * D], w0[2 * D:]
    qmf = np.asarray(query_mask).astype(f)                  # (B, LQ)
    idx, NC2 = _aux(context_mask)
    NT = NC2 // 128

    sq = query @ wq                                         # (B, LQ)
    sq_bias = ((1.0 - qmf) * sq + qmf * NEG).astype(f)      # -1e30 on masked q
    qwmT = (query * wm).transpose(0, 2, 1)                  # (B, D, LQ) f32
    qe = np.zeros((B, LQ, 513), f)
    qe[:, :, 0:256] = query
    qe[:, :, 512] = 1.0

    in_maps = []
    for c in range(NCORES):
        m = {"ctxT2": np.zeros((BG, 128, 2, NKD, NC2), NPBF16),
             "ctx2": np.zeros((BG, 128, 2, NT, 258), NPBF16),
             "qwm2": np.empty((BG, 128, 2, NKD, LQ), NPBF16),
             "fvec": np.zeros((BG, 128, 2, NQT), f),
             "mq": np.zeros((BG, 128, 2, NQT), NPBF16),
             "qe2": np.empty((BPC, 128, NQT, 513), NPBF16)}
        for lb in range(BPC):
            b = c * BPC + lb
            g, jj = divmod(lb, 2)
            U = idx[b]
            n = len(U)
            cU = context[b][U]                              # (n, D)
            scU = cU @ wc                                   # (n,)
            ctxT_pad = np.zeros((D, NC2), f)
            ctxT_pad[:, :n] = cU.T
            m["ctxT2"][g, :, jj] = _img(ctxT_pad).astype(NPBF16)
            # ctx rows scaled by e^{sc[c]} (column-softmax weight); the
            # ones-col picks up the same factor => correct normalizer.
            ctx_pad = np.zeros((NC2, 258), f)
            ctx_pad[:n, :D] = cU
            ctx_pad[:n, D] = 1.0
            ctx_pad[:n] *= np.exp(scU, dtype=f)[:, None]
            m["ctx2"][g, :, jj] = _img(ctx_pad).astype(NPBF16)
            m["qwm2"][g, :, jj] = _img(qwmT[b]).astype(NPBF16)
            m["fvec"][g, :, jj] = sq_bias[b].reshape(NQT, 128).T
            m["mq"][g, :, jj] = (1.0 - qmf[b]).reshape(NQT, 128).T
            m["qe2"][lb] = _img(qe[b]).astype(NPBF16)
        in_maps.append(m)
    return in_maps


_cached_nc = {}


def get_program(NC2):
    if NC2 not in _cached_nc:
        _cached_nc[NC2] = build_program(NC2)
    return _cached_nc[NC2]


def run_on_hw(in_maps, **kwargs):
    NC2 = in_maps[0]["ctxT2"].shape[-1]
    nc = get_program(NC2)
    return run_bass_kernel_spmd(nc, in_maps, core_ids=list(range(NCORES)),
                                **kwargs)


def kernel(context, query, context_mask, query_mask, w0):
    f = np.float32
    context = np.asarray(context, dtype=f)
    query = np.asarray(query, dtype=f)
    w0 = np.asarray(w0, dtype=f)
    qmask = np.asarray(query_mask).astype(bool)
    idx, NC2 = _aux(context_mask)
    NT = NC2 // 128
    ctxmean = context.mean(1, dtype=np.float64).astype(f)   # (B, D)
    in_maps = host_prep(context, query, context_mask, query_mask, w0)
    res = run_on_hw(in_maps)

    A = np.empty((B, LC, D), f)
    Bm = np.empty((B, LC, D), f)
    cmask = np.asarray(context_mask).astype(bool)
    for c in range(NCORES):
        r = res.results[c]
        for lb in range(BPC):
            b = c * BPC + lb
            U = idx[b]
            n = len(U)
            ABr = r["ABo"][lb].astype(f).reshape(128, NT, 512).swapaxes(0, 1)
            ABr = ABr.reshape(NC2, 512)
            rs = r["rso"][lb][:, :NT].astype(f).T.reshape(NC2)
            cm = r["cmo"][lb][0].astype(f)                  # (256,)
            inv = 1.0 / rs[:n]
            A[b][U] = ABr[:n, 0:256] * inv[:, None]
            Bm[b][U] = ABr[:n, 256:512] * inv[:, None]
            nmq = float(qmask[b].sum())
            colmean = (cm + nmq * ctxmean[b]) / np.float32(LQ)
            mrow = cmask[b]
            A[b][mrow] = query[b].mean(0, dtype=np.float64).astype(f)
            Bm[b][mrow] = colmean
    return A, Bm
